# revision 50
# baseline (speedup 1.0000x reference)
"""Causal self-attention Trainium2 kernel (8 NeuronCores, SPMD).

Problem: B=4, T=2048, C=1024, H=16 heads, D=64.
  qkv = x @ w_attn + b_attn ; causal softmax attention ; out = y @ w_proj + b_proj

Sharding: core c = 2*b + g  handles batch b with head-group g (heads 8g..8g+7).
Each core computes a partial projection output (its 8 heads' contribution);
the host sums the two partials per batch and adds b_proj.

Per-core layouts (all fp32 in HBM, consumed as float32r by the PE):
  xT  [1024, 2048]  = x[b].T                       (contraction dim on partitions)
  w_q/w_k/w_v [1024, 512], b_q/b_k/b_v [512]       (head-group slices of w_attn)
  w_p [512, 1024]                                  (head-group rows of w_proj)
  mask [128, 896]   mask[tk, m] = 1 if m >= tk+384 (sliced per diagonal offset)

On-chip:
  qT, kT  [128, 4, 2048]  (d-group, t)  - head pair per 128 partitions
  v       [128, 16, 520]  (t-tile, 8 heads x (64 v | 1 ones))  ones col -> denominator
  sT tile [128 tk, 512 tq] = kT.T @ qT ; p = exp(s/8) (ScalarE, scale folded)
  y_psum  [65, 512] accumulates [v|1].T @ p over kt tiles; row 64 = softmax denom
  yT      [128, 4, 2048]  normalized via reciprocal + gpsimd partition_broadcast
  out     [2048, 1024] = yT.T @ w_p  (partial; host adds pair + b_proj)
"""

import sys
import os

sys.path.insert(0, "/opt/trn_rl_repo")

import numpy as np
import ml_dtypes

import concourse.bass as bass
import concourse.mybir as mybir
import concourse.tile as tile
from concourse.vector_clock import ScopedClock
from concourse.bass_utils import run_bass_kernel_spmd

F32 = mybir.dt.float32
F32R = mybir.dt.float32r
BF16 = mybir.dt.bfloat16
EXP = mybir.ActivationFunctionType.Exp

B, T, C, H = 4, 2048, 1024, 16
D = C // H            # 64
NH = 8                # local heads per core
DG = 4                # d-groups of 128 partitions (2 heads each)
CK = 8                # contraction chunks of 128 over C
NQ = 4                # q tiles of 512
NT = 16               # t tiles of 128
QW = 512              # q tile width
KW = 128              # k tile width (partition dim of sT)
VW = D + 1            # v block incl ones column


# ---------------------------------------------------------------------------
# Tile compatibility patches for this walrus build: it accepts at most ONE
# sync wait per instruction, while TileContext attaches several.  Split the
# extras onto dedicated nops (same engine, just before the instruction).
# ---------------------------------------------------------------------------
def _install_patches():
    if getattr(tile.TileContext, "_wsplit_patched", False):
        return

    def _drain_and_barrier(self, tick_clock, wait_clock):
        drain_inst = self.nc.sync.drain()
        wait_clock.add_sem_waits(
            drain_inst.ins, ScopedClock({None: tick_clock.global_clock})
        )
        si = drain_inst.ins.sync_info
        waits = list(si.on_wait or []) if si is not None else []
        if len(waits) > 1:
            si.on_wait = waits[:1]
            for w in waits[1:]:
                n = self.nc.sync.nop(nofuse=True, hint="tail_wait")
                if n.ins.sync_info is None:
                    n.ins.sync_info = mybir.SyncInfo(on_wait=[w], on_update=[])
                else:
                    n.ins.sync_info.on_wait = [w]
        self.nc.all_engine_barrier()
        popped = self.nc._tile_sem_poison_stack.pop()
        assert popped is self._sem_poison
        self.nc.clear_and_free_semaphores(list(self.sems.allocated().values()))
        self.nc.all_engine_barrier()

    _orig_commit = tile.TileContext._commit_and_lower

    def _commit_and_lower(self, inst, original_block, old_bb_map, bb_to_exit_bb):
        si = getattr(inst, "sync_info", None)
        if si is not None and si.on_wait and len(si.on_wait) > 1:
            waits = list(si.on_wait)
            si.on_wait = [waits[-1]]
            for w in waits[:-1]:
                nop = self.nc.engines[inst.engine].nop(nofuse=True, hint="wsplit")
                if nop.ins.sync_info is None:
                    nop.ins.sync_info = mybir.SyncInfo(on_wait=[w], on_update=[])
                else:
                    nop.ins.sync_info.on_wait = [w]
        return _orig_commit(self, inst, original_block, old_bb_map, bb_to_exit_bb)

    tile.TileContext._drain_and_barrier = _drain_and_barrier
    tile.TileContext._commit_and_lower = _commit_and_lower
    tile.TileContext._wsplit_patched = True


# ---------------------------------------------------------------------------
# Kernel program
# ---------------------------------------------------------------------------
def _build_program():
    _install_patches()
    nc = bass.Bass()

    xT_e = nc.dram_tensor("xT", [C, T], BF16, kind="ExternalInput")
    wq_e = nc.dram_tensor("wq", [C, NH * D], BF16, kind="ExternalInput")
    wk_e = nc.dram_tensor("wk", [C, NH * D], BF16, kind="ExternalInput")
    wv_e = nc.dram_tensor("wv", [C, NH * D], BF16, kind="ExternalInput")
    bq_e = nc.dram_tensor("bq", [NH * D], F32, kind="ExternalInput")
    bk_e = nc.dram_tensor("bk", [NH * D], F32, kind="ExternalInput")
    wp_e = nc.dram_tensor("wp", [NH * D, C], BF16, kind="ExternalInput")
    mask_e = nc.dram_tensor("mask", [KW, 896], BF16, kind="ExternalInput")
    sel_e = nc.dram_tensor("sel", [NH, NH * D], BF16, kind="ExternalInput")
    eye_e = nc.dram_tensor("eye", [KW, KW + 1], BF16, kind="ExternalInput")
    out_e = nc.dram_tensor("out", [T, C], F32, kind="ExternalOutput")

    with tile.TileContext(nc) as tc:
        with (
            tc.tile_pool(name="persist", bufs=1) as persist,
            tc.tile_pool(name="psum_y", bufs=2, space="PSUM") as psum_y,
        ):
            # causal mask as additive bias (0 visible / -480 masked),
            # applied on the PE by accumulating eye.T @ mask into sT psum.
            mask_sb = persist.tile([KW, 896], BF16, tag="mask")
            nc.sync.dma_start(mask_sb[:], mask_e[:])
            # eye cols 0:128 = identity (causal-bias matmul lhsT);
            # col 128 = all-ones (v ones-column source)
            eye_sb = persist.tile([KW, KW + 1], BF16, tag="eye")
            nc.sync.dma_start(eye_sb[:], eye_e[:])
            # selector kron(I8, ones64): bc matmul picks head h's recip row
            sel_sb = persist.tile([NH, NH * D], BF16, tag="sel")
            nc.sync.dma_start(sel_sb[:], sel_e[:])
            # q/k biases in column layout [d_in_group, dg] for the fused
            # bias-add on the ScalarE psum->sbuf copy (per-partition bias).
            bqk_sb = persist.tile([128, 2 * DG], F32, tag="bqk")
            nc.sync.dma_start(
                bqk_sb[:, 0:DG], bq_e.rearrange("(dg p) -> p dg", p=128)
            )
            nc.sync.dma_start(
                bqk_sb[:, DG : 2 * DG], bk_e.rearrange("(dg p) -> p dg", p=128)
            )

            qT = persist.tile([128, DG, T], BF16, tag="qT")
            kT = persist.tile([128, DG, T], BF16, tag="kT")
            v_sb = persist.tile([128, NT, NH * VW], BF16, tag="v")

            # ---------------- Phase 1: QKV projections -------------------
            IDENT = mybir.ActivationFunctionType.Identity
            SW = 512  # x slab width (double-buffered)
            NS = T // SW
            with (
                tc.tile_pool(name="ph1", bufs=1) as ph1,
                tc.tile_pool(name="ph1x", bufs=2) as ph1x,
                tc.tile_pool(name="ph1ps", bufs=3, space="PSUM") as ph1ps,
                tc.tile_pool(name="ph1psv", bufs=2, space="PSUM") as ph1psv,
            ):
                wq_sb = ph1.tile([128, CK, NH * D], BF16, tag="wq")
                wk_sb = ph1.tile([128, CK, NH * D], BF16, tag="wk")
                wv_sb = ph1.tile([128, CK, NH * D], BF16, tag="wv")
                for w_sb, w_ext in ((wq_sb, wq_e), (wk_sb, wk_e), (wv_sb, wv_e)):
                    nc.sync.dma_start(
                        w_sb[:],
                        w_ext.rearrange("(ck p) d -> p ck d", p=128),
                    )

                for th in range(NS):
                    xt = ph1x.tile([128, CK, SW], BF16, tag="xt")
                    nc.sync.dma_start(
                        xt[:],
                        xT_e[:, th * SW : (th + 1) * SW]
                        .rearrange("(ck p) t -> p ck t", p=128),
                    )

                    # q and k (transposed layout [d, t]); bias fused into the
                    # ScalarE psum->sbuf copy (per-partition bias add).
                    for dst, w_sb, bc0 in ((qT, wq_sb, 0), (kT, wk_sb, DG)):
                        for dg in range(DG):
                            ps = ph1ps.tile([128, SW], F32, tag="ph1ps")
                            for ck in range(CK):
                                nc.tensor.matmul(
                                    ps[:],
                                    w_sb[:, ck, dg * 128 : (dg + 1) * 128],
                                    xt[:, ck, :],
                                    start=(ck == 0),
                                    stop=(ck == CK - 1),
                                )
                            nc.scalar.activation(
                                dst[:, dg, th * SW : th * SW + SW],
                                ps[:],
                                IDENT,
                                bias=bqk_sb[:, bc0 + dg : bc0 + dg + 1],
                            )

                    # v (natural layout [t, d]) + ones column; bv is folded
                    # into the host-side output constant (softmax weights
                    # sum to 1, so + bv commutes through the attention)
                    for tt in range(SW // 128):
                        tta = th * (SW // 128) + tt
                        ps = ph1psv.tile([128, NH * D], F32, tag="ph1psv")
                        for ck in range(CK):
                            nc.tensor.matmul(
                                ps[:],
                                xt[:, ck, tt * 128 : (tt + 1) * 128],
                                wv_sb[:, ck, :],
                                start=(ck == 0),
                                stop=(ck == CK - 1),
                            )
                        v_blk = v_sb[:, tta, :].rearrange("p (h e) -> p h e", e=VW)
                        nc.vector.tensor_copy(
                            v_blk[:, :, 0:D],
                            ps[:].rearrange("p (h d) -> p h d", d=D),
                        )
                        nc.vector.tensor_copy(
                            v_blk[:, :, D : D + 1],
                            eye_sb[:, KW : KW + 1].broadcast_to([KW, NH, 1]),
                        )

            # ------- Phase 2 + 3: attention, proj interleaved per qt ------
            yT = persist.tile([128, DG, T], BF16, tag="yT")
            with (
                tc.tile_pool(name="ph2", bufs=3) as ph2,
                tc.tile_pool(name="ph2ps", bufs=2, space="PSUM") as ph2ps,
                tc.tile_pool(name="ph2bc", bufs=2, space="PSUM") as ph2bc,
                tc.tile_pool(name="ph3", bufs=1) as ph3,
                tc.tile_pool(name="work", bufs=3) as work,
            ):
                wp_sb = ph3.tile([128, DG, C], BF16, tag="wp")
                nc.sync.dma_start(
                    wp_sb[:], wp_e.rearrange("(jc p) e -> p jc e", p=128)
                )
                IDENT2 = mybir.ActivationFunctionType.Identity

                def norm_and_proj(qt, den_all):
                    # one batched reciprocal for all 8 heads of this qt
                    recip_all = ph2.tile([NH, QW], BF16, tag="recip", bufs=2)
                    with nc.allow_low_precision(reason="softmax reciprocal"):
                        nc.vector.reciprocal(recip_all[:], den_all[:])
                    for h in range(NH):
                        dg = h // 2
                        po = 64 * (h % 2)
                        bc_slot = ph2bc.tile([KW, QW], F32, tag="bcops")
                        bc_ps = bc_slot[0:D, 0:QW]
                        nc.tensor.matmul(
                            bc_ps,
                            sel_sb[:, h * D : (h + 1) * D],
                            recip_all[:],
                            start=True,
                            stop=True,
                        )
                        ysl = yT[po : po + 64, dg, qt * QW : (qt + 1) * QW]
                        nc.vector.tensor_mul(ysl, ysl, bc_ps)
                    # output projection for this qt's t range
                    for tt in range(4 * qt, 4 * (qt + 1)):
                        for eh in range(2):
                            o_slot = ph2bc.tile([KW, QW], F32, tag="bcops")
                            o_ps = o_slot[:, 0:QW]
                            for jc in range(DG):
                                nc.tensor.matmul(
                                    o_ps,
                                    yT[:, jc, tt * 128 : (tt + 1) * 128],
                                    wp_sb[:, jc, eh * QW : (eh + 1) * QW],
                                    start=(jc == 0),
                                    stop=(jc == DG - 1),
                                )
                            o_sb = work.tile([128, QW], F32, tag="osb")
                            nc.vector.tensor_copy(o_sb[:], o_ps)
                            nc.sync.dma_start(
                                out_e[
                                    tt * 128 : (tt + 1) * 128,
                                    eh * QW : (eh + 1) * QW,
                                ],
                                o_sb[:],
                            )

                pending_qt = None
                for qt in range(NQ):
                    den_all = ph2.tile([NH, QW], F32, tag="den", bufs=2)
                    nkt = 4 * (qt + 1)
                    for hp in range(NH // 2):
                        # head pair: hA on partitions 0:64, hB on 64:128;
                        # one s psum tile per kt packs A | B column halves so
                        # a single exp covers both and the A/B matmuls are
                        # adjacent (PE half-array row tiles 0 / 64).
                        dg = hp
                        hA, hB = 2 * hp, 2 * hp + 1
                        qA = qT[0:64, dg, qt * QW : (qt + 1) * QW]
                        qB = qT[64:128, dg, qt * QW : (qt + 1) * QW]
                        y_psA = psum_y.tile([D + 1, QW], F32, tag="yps")
                        y_psB = psum_y.tile([D + 1, QW], F32, tag="yps")

                        def emit_y(kt, p_t):
                            # p is exactly 0 in fully-masked cols, so skip
                            # them: accumulate only cols o:QW.  stop flags are
                            # sim-only bookkeeping (skip_group_check).
                            o = max(KW * kt - QW * qt, 0)
                            for h, y_ps, c0 in ((hA, y_psA, 0), (hB, y_psB, QW)):
                                nc.tensor.matmul(
                                    y_ps[:, o:QW],
                                    v_sb[:, kt, h * VW : (h + 1) * VW],
                                    p_t[:, c0 + o : c0 + QW],
                                    start=(kt == 0),
                                    stop=(kt == nkt - 1),
                                    skip_group_check=True,
                                )

                        pending = []
                        for kt in range(nkt):
                            ksl = slice(kt * KW, (kt + 1) * KW)
                            off = KW * kt - QW * qt
                            diag = off >= 0
                            o = max(off, 0)
                            sp = ph2ps.tile([KW, 2 * QW], F32, tag="sps")
                            # s over visible q columns only; A/B adjacent
                            nc.tensor.matmul(
                                sp[0:KW, o:QW],
                                kT[0:64, dg, ksl],
                                qA[:, o:QW],
                                start=True,
                                stop=True,
                            )
                            nc.tensor.matmul(
                                sp[0:KW, QW + o : 2 * QW],
                                kT[64:128, dg, ksl],
                                qB[:, o:QW],
                                start=True,
                                stop=True,
                            )
                            if diag:
                                # -480 causal bias on the 128-wide triangle
                                # only (cols o:o+128); hw accumulates onto s
                                for c0 in (0, QW):
                                    nc.tensor.matmul(
                                        sp[:, c0 + o : c0 + o + KW],
                                        eye_sb[:, 0:KW],
                                        mask_sb[:, 384:512],
                                        start=False,
                                        stop=True,
                                        skip_group_check=True,
                                    )
                            pt = ph2.tile([KW, 2 * QW], BF16, tag="pt", bufs=6)
                            if o > 0:
                                # fully-masked q cols [0:o) are never read
                                # (AV + exp both skip them) — no memset needed
                                nc.scalar.activation(
                                    pt[:, o:QW], sp[:, o:QW], EXP, scale=0.125
                                )
                                nc.scalar.activation(
                                    pt[:, QW + o : 2 * QW],
                                    sp[:, QW + o : 2 * QW],
                                    EXP,
                                    scale=0.125,
                                )
                            else:
                                nc.scalar.activation(pt[:], sp[:], EXP, scale=0.125)
                            # software pipeline: y matmuls lag two iterations
                            # so PE never blocks on in-flight exps
                            pending.append((kt, pt))
                            if len(pending) > 2:
                                emit_y(*pending.pop(0))
                        for pend in pending:
                            emit_y(*pend)
                        # spill unnormalized y straight into yT (freeing the
                        # psum slots); normalization is applied in-place later
                        for h, y_ps in ((hA, y_psA), (hB, y_psB)):
                            po = 64 * (h % 2)
                            ysl = yT[po : po + 64, dg, qt * QW : (qt + 1) * QW]
                            nc.vector.tensor_copy(ysl, y_ps[0:D, :])
                            den_st = ph2.tile([1, QW], F32, tag="denst", bufs=2)
                            nc.vector.tensor_copy(den_st[:], y_ps[D : D + 1, :])
                            # spread den rows across partitions 0..7 (DMA can
                            # write arbitrary partitions; engines cannot)
                            nc.sync.dma_start(den_all[h : h + 1, :], den_st[:])

                    # qt-level software pipeline: normalize+proj of the
                    # previous qt runs while this qt's attention streams
                    if pending_qt is not None:
                        norm_and_proj(*pending_qt)
                    pending_qt = (qt, den_all)
                norm_and_proj(*pending_qt)
    return nc


_CACHE = {}
last_exec_time_ns = None


def _causal_mask_np():
    # additive bias: 0 where visible (m >= tk + 384), -480 where masked
    m = np.full((KW, 896), -480.0, dtype=np.float32)
    tk = np.arange(KW)[:, None]
    mm = np.arange(896)[None, :]
    m[mm >= tk + 384] = 0.0
    return m


def kernel(x, w_attn, b_attn, w_proj, b_proj):
    global last_exec_time_ns
    x = np.asarray(x, dtype=np.float32)
    w_attn = np.asarray(w_attn, dtype=np.float32)
    b_attn = np.asarray(b_attn, dtype=np.float32)
    w_proj = np.asarray(w_proj, dtype=np.float32)
    b_proj = np.asarray(b_proj, dtype=np.float32)

    if "nc" not in _CACHE:
        _CACHE["nc"] = _build_program()
    nc = _CACHE["nc"]

    bf16 = ml_dtypes.bfloat16
    mask = _causal_mask_np().astype(bf16)
    in_maps = []
    for c in range(8):
        b, g = divmod(c, 2)
        s = slice(g * 512, (g + 1) * 512)
        in_maps.append(
            {
                "xT": np.ascontiguousarray(x[b].T).astype(bf16),
                "wq": np.ascontiguousarray(w_attn[:, s]).astype(bf16),
                "wk": np.ascontiguousarray(w_attn[:, 1024 + g * 512 : 1024 + (g + 1) * 512]).astype(bf16),
                "wv": np.ascontiguousarray(w_attn[:, 2048 + g * 512 : 2048 + (g + 1) * 512]).astype(bf16),
                "bq": np.ascontiguousarray(b_attn[s]),
                "bk": np.ascontiguousarray(b_attn[1024 + g * 512 : 1024 + (g + 1) * 512]),
                "wp": np.ascontiguousarray(w_proj[s, :]).astype(bf16),
                "mask": mask,
                "sel": np.kron(np.eye(NH, dtype=np.float32), np.ones((1, D), np.float32)).astype(bf16),
                "eye": np.concatenate(
                    [np.eye(KW, dtype=np.float32), np.ones((KW, 1), np.float32)],
                    axis=1,
                ).astype(bf16),
            }
        )

    res = run_bass_kernel_spmd(nc, in_maps, core_ids=list(range(8)))
    last_exec_time_ns = res.exec_time_ns

    # bv is not applied on-device: softmax weights sum to 1, so the v bias
    # contributes exactly bv @ w_proj to every output row — fold it (and
    # b_proj) into one host-side constant.
    b_eff = b_proj + b_attn[2048:3072] @ w_proj
    out = np.empty((B, T, C), dtype=np.float32)
    for b in range(B):
        out[b] = res.results[2 * b]["out"] + res.results[2 * b + 1]["out"] + b_eff
    return out

